# revision 14
# baseline (speedup 1.0000x reference)
"""Trainium2 Bass kernel for nn_AttentionModule (B=4, C=512, N=4096, CQK=64).

Sharding: 8 cores = (batch b, query-half h). Each core receives x[b] with
columns rotated so that its 2048-query slab is always columns 0:2048 —
attention output for query i depends on the full key set but is invariant
to key permutation, so rotation keeps the program identical across cores.

Per-core pipeline (all on one NeuronCore):
  A) stream x (split across SWDGE-cast and HWDGE+DVE-cast paths), project
     k = Wk x + bk (f32r), q (slab only), and vT[j, c] = (x^T Wv^T)*gamma
     + gamma*bv (produced directly transposed -> no on-chip transposes),
     stored bf16.
  B) per 512-query block: 16 logitsT[j, i] = k^T q matmuls (f32r, j on
     partitions) into 2-bank PSUM groups, one exp per group on ACT -> bf16
     E arena [128, 16384]; denominator = pairwise halving adds (bf16 tree,
     non-destructive level 1) + ones[128,128] matmul (K=128 partition
     reduce); AV accumulated over 32 j-tiles in PSUM (bf16), c-outer with
     rotated j order so each av[c] finishes as its exps land; out =
     AV * recip + x on DVE, emitted inline as each av[c] completes.
"""

import sys

if "/opt/trn_rl_repo" not in sys.path:
    sys.path.insert(0, "/opt/trn_rl_repo")

from contextlib import ExitStack

import numpy as np

import concourse.tile as tile
from concourse import bacc, mybir
from concourse.bass_utils import run_bass_kernel_spmd

B, C, N = 4, 512, 4096
CQK = C // 8
NCORES = 8
SLAB = N // 2            # queries per core
CHUNK = 512              # matmul moving free dim
NCHUNK = N // CHUNK      # 8 column chunks of x
NKT = C // 128           # 4 contraction tiles over input channels
NJT = N // 128           # 32 key tiles
NBLK = SLAB // CHUNK     # 4 query blocks per core
JG = 2                   # j-tiles per logits/exp group
NG = NJT // JG           # 16 groups per block

F32 = mybir.dt.float32
F32R = mybir.dt.float32r
BF16 = mybir.dt.bfloat16

_compiled = None


def _build():
    nc = bacc.Bacc("TRN2", debug=False, num_devices=NCORES)

    x_d = nc.dram_tensor("x", [C, N], F32, kind="ExternalInput").ap()
    wqT_d = nc.dram_tensor("wqT", [C, CQK], F32, kind="ExternalInput").ap()
    wkT_d = nc.dram_tensor("wkT", [C, CQK], F32, kind="ExternalInput").ap()
    wvT_d = nc.dram_tensor("wvT", [C, C], F32, kind="ExternalInput").ap()
    bq_d = nc.dram_tensor("bq", [CQK, 1], F32, kind="ExternalInput").ap()
    bk_d = nc.dram_tensor("bk", [CQK, 1], F32, kind="ExternalInput").ap()
    bvg_d = nc.dram_tensor("bvg", [128, C], F32, kind="ExternalInput").ap()
    ones_d = nc.dram_tensor("ones", [128, 128], F32, kind="ExternalInput").ap()
    out_d = nc.dram_tensor("out", [C, SLAB], F32, kind="ExternalOutput").ap()

    with tile.TileContext(nc) as tc, ExitStack() as ctx:
        consts = ctx.enter_context(tc.tile_pool(name="consts", bufs=1))
        xs_pool = ctx.enter_context(tc.tile_pool(name="xs", bufs=8))
        xf_pool = ctx.enter_context(tc.tile_pool(name="xf", bufs=4))
        qk_pool = ctx.enter_context(tc.tile_pool(name="qk", bufs=1))
        vt_pool = ctx.enter_context(tc.tile_pool(name="vt", bufs=NJT))
        e_pool = ctx.enter_context(tc.tile_pool(name="e", bufs=2))
        sc_pool = ctx.enter_context(tc.tile_pool(name="sc", bufs=1))
        sm_pool = ctx.enter_context(tc.tile_pool(name="sm", bufs=2))
        xr_pool = ctx.enter_context(tc.tile_pool(name="xr", bufs=2))
        o_pool = ctx.enter_context(tc.tile_pool(name="o", bufs=2))
        big_ps = ctx.enter_context(tc.tile_pool(name="bigps", bufs=2, space="PSUM"))
        av_ps = ctx.enter_context(tc.tile_pool(name="avps", bufs=4, space="PSUM"))

        # --- constants (combined single-DMA weight loads) ---
        wk_all = consts.tile([128, NKT * CQK], F32R, tag="wk")
        wq_all = consts.tile([128, NKT * CQK], F32R, tag="wq")
        wv_all = consts.tile([128, NKT * CHUNK], F32R, tag="wv")
        bq = consts.tile([CQK, 1], F32, tag="bq")
        bk = consts.tile([CQK, 1], F32, tag="bk")
        bvg = consts.tile([128, C], F32, tag="bvg")
        ones = consts.tile([128, 128], BF16, tag="ones")
        nc.gpsimd.dma_start(wk_all[:].rearrange("p (k c) -> p k c", k=NKT),
                            wkT_d.rearrange("(k p) c -> p k c", k=NKT))
        nc.gpsimd.dma_start(wq_all[:].rearrange("p (k c) -> p k c", k=NKT),
                            wqT_d.rearrange("(k p) c -> p k c", k=NKT))
        nc.sync.dma_start(bq[:], bq_d[:])
        nc.sync.dma_start(bk[:], bk_d[:])
        wk = [wk_all[:, k * CQK : (k + 1) * CQK] for k in range(NKT)]
        wq = [wq_all[:, k * CQK : (k + 1) * CQK] for k in range(NKT)]
        wv = [wv_all[:, k * CHUNK : (k + 1) * CHUNK] for k in range(NKT)]

        # low half (partitions 0:64) written by projections; high half is a
        # DMA copy so logits matmuls can row-pack two j-tiles per PE pass
        q_sb = qk_pool.tile([128, SLAB], F32R, tag="q")
        k_sb = qk_pool.tile([128, N], F32R, tag="k")
        vt = []  # 32 tiles [128 j, 512 c] bf16

        # --- phase A: projections ---
        for ch in range(NCHUNK):
            cols = slice(ch * CHUNK, (ch + 1) * CHUNK)
            xt = []
            for k in range(NKT):
                t = xs_pool.tile([128, CHUNK], F32R, tag="xs")
                # chunk 0 entirely via HWDGE so PE start waits only on wk;
                # later chunks split across SWDGE-cast and HWDGE+DVE-cast
                if ch > 0 and k % 2 == 0:
                    nc.gpsimd.dma_start(t[:], x_d[k * 128 : (k + 1) * 128, cols])
                else:
                    tf = xf_pool.tile([128, CHUNK], F32, tag="xf")
                    nc.sync.dma_start(tf[:], x_d[k * 128 : (k + 1) * 128, cols])
                    nc.vector.tensor_copy(t[:], tf[:])
                xt.append(t)
            if ch == 0:
                nc.sync.dma_start(bvg[:], bvg_d[:])
                nc.gpsimd.dma_start(ones[:], ones_d[:])
                # wv is first needed by the vT matmuls of chunk 0; loading it
                # here keeps the k/q projections' critical path short
                nc.gpsimd.dma_start(
                    wv_all[:].rearrange("p (k c) -> p k c", k=NKT),
                    wvT_d.rearrange("(k p) c -> p k c", k=NKT))

            k_ps = av_ps.tile([CQK, CHUNK], F32, tag="av")
            for k in range(NKT):
                nc.tensor.matmul(k_ps[:], wk[k], xt[k][:],
                                 start=(k == 0), stop=(k == NKT - 1))
            nc.vector.tensor_scalar_add(k_sb[0:CQK, cols], k_ps[:], bk[:])
            nc.sync.dma_start(k_sb[CQK:128, cols], k_sb[0:CQK, cols])

            if ch < NBLK:
                q_ps = av_ps.tile([CQK, CHUNK], F32, tag="av")
                for k in range(NKT):
                    nc.tensor.matmul(q_ps[:], wq[k], xt[k][:],
                                     start=(k == 0), stop=(k == NKT - 1))
                nc.vector.tensor_scalar_add(q_sb[0:CQK, cols], q_ps[:], bq[:])
                nc.sync.dma_start(q_sb[CQK:128, cols], q_sb[0:CQK, cols])

            for jt in range(4):
                jcols = slice(jt * 128, (jt + 1) * 128)
                v_ps = av_ps.tile([128, C], F32, tag="av")
                for k in range(NKT):
                    nc.tensor.matmul(v_ps[:], xt[k][:, jcols], wv[k],
                                     start=(k == 0), stop=(k == NKT - 1))
                v_t = vt_pool.tile([128, C], BF16, tag="vt")
                nc.vector.tensor_add(v_t[:], v_ps[:], bvg[:])
                vt.append(v_t)

        # --- phase B: attention per query block ---
        H = NJT * CHUNK // 2  # arena half width (8192)
        for blk in range(NBLK):
            icols = slice(blk * CHUNK, (blk + 1) * CHUNK)
            av = [av_ps.tile([128, CHUNK], F32, tag="av", name=f"av{blk}_{i}")
                  for i in range(NKT)]
            arena = e_pool.tile([128, NJT * CHUNK], BF16, tag="arena")
            scratch = sc_pool.tile([128, H], BF16, tag="scratch")

            # logits + exp, all groups up front: PE stays fed while ACT/DVE
            # trail behind on exps and the exp-sum tree
            for g in range(NG):
                l_ps = big_ps.tile([128, JG * CHUNK], F32, tag="big")
                for j in range(JG):
                    jt = g * JG + j
                    # row-pack: even j-tile on array rows 0-63, odd on 64-127;
                    # the two matmuls execute concurrently in the PE array
                    lo, hi = (0, CQK) if j % 2 == 0 else (CQK, 128)
                    nc.tensor.matmul(l_ps[:, j * CHUNK : (j + 1) * CHUNK],
                                     k_sb[lo:hi, jt * 128 : (jt + 1) * 128],
                                     q_sb[lo:hi, icols], start=True, stop=True,
                                     tile_position=(lo, 0))
                nc.scalar.activation(arena[:, g * JG * CHUNK : (g + 1) * JG * CHUNK],
                                     l_ps[:], mybir.ActivationFunctionType.Exp)
                with nc.allow_low_precision(reason="bf16 pairwise exp-sum tree"):
                    if g == NG // 2 - 1:
                        nc.vector.tensor_add(scratch[:, 0 : H // 2],
                                             arena[:, 0 : H // 2],
                                             arena[:, H // 2 : H])
                    elif g == NG - 1:
                        nc.vector.tensor_add(scratch[:, H // 2 : H],
                                             arena[:, H : H + H // 2],
                                             arena[:, H + H // 2 :])

            # finish the halving tree (in place on scratch)
            with nc.allow_low_precision(reason="bf16 pairwise exp-sum tree"):
                w = H // 2
                while w >= CHUNK:
                    nc.vector.tensor_add(scratch[:, 0:w], scratch[:, 0:w],
                                         scratch[:, w : 2 * w])
                    w //= 2

            recip = sm_pool.tile([128, CHUNK], F32, tag="recip", name=f"rc{blk}")

            def norm_c(c, blk=blk, av=av, icols=icols, recip=recip):
                rows = slice(c * 128, (c + 1) * 128)
                xres = xr_pool.tile([128, CHUNK], F32, tag="xr", name=f"xr{blk}_{c}")
                nc.sync.dma_start(xres[:], x_d[rows, icols])
                t = o_pool.tile([128, CHUNK], F32, tag="om", name=f"om{blk}_{c}")
                nc.vector.tensor_mul(t[:], av[c][:], recip[:])
                o = o_pool.tile([128, CHUNK], F32, tag="oo", name=f"oo{blk}_{c}")
                nc.vector.tensor_add(o[:], t[:], xres[:])
                nc.sync.dma_start(out_d[rows, icols], o[:])

            # AV: c-outer, j rotated so av[c]'s last matmul needs an early exp
            for c in range(NKT):
                for t in range(NJT):
                    jt = (c * (NJT // NKT) + t) % NJT
                    nc.tensor.matmul(av[c][:],
                                     vt[jt][:, c * 128 : (c + 1) * 128],
                                     arena[:, jt * CHUNK : (jt + 1) * CHUNK],
                                     start=(t == 0), stop=(t == NJT - 1))
                if c == 1:
                    # denominator: reduce over partitions, broadcast to all
                    s_ps = big_ps.tile([128, CHUNK], F32, tag="big",
                                       name=f"sps{blk}")
                    nc.tensor.matmul(s_ps[:], ones[:], scratch[:, 0:CHUNK],
                                     start=True, stop=True)
                    nc.vector.reciprocal(recip[:], s_ps[:])
                elif c == 2:
                    norm_c(0)
                elif c == 3:
                    norm_c(1)
            norm_c(2)
            norm_c(3)

    nc.compile()
    return nc


def _get_compiled():
    global _compiled
    if _compiled is None:
        _compiled = _build()
    return _compiled


def kernel(x, Wq, bq, Wk, bk, Wv, bv, gamma, **run_kwargs):
    x = np.asarray(x, dtype=np.float32)
    Wq = np.asarray(Wq, dtype=np.float32)
    bq = np.asarray(bq, dtype=np.float32)
    Wk = np.asarray(Wk, dtype=np.float32)
    bk = np.asarray(bk, dtype=np.float32)
    Wv = np.asarray(Wv, dtype=np.float32)
    bv = np.asarray(bv, dtype=np.float32)
    g = float(np.asarray(gamma).reshape(-1)[0])

    shared = {
        "wqT": np.ascontiguousarray(Wq.T),
        "wkT": np.ascontiguousarray(Wk.T),
        "wvT": np.ascontiguousarray(Wv.T * g),
        "bq": np.ascontiguousarray(bq.reshape(CQK, 1)),
        "bk": np.ascontiguousarray(bk.reshape(CQK, 1)),
        "bvg": np.ascontiguousarray(np.tile((bv * g).reshape(1, C), (128, 1))),
        "ones": np.ones((128, 128), dtype=np.float32),
    }
    in_maps = []
    for core in range(NCORES):
        b, h = divmod(core, 2)
        xb = x[b]
        if h:
            xb = np.concatenate([xb[:, SLAB:], xb[:, :SLAB]], axis=1)
        in_maps.append({"x": np.ascontiguousarray(xb), **shared})

    nc = _get_compiled()
    res = run_bass_kernel_spmd(nc, in_maps, core_ids=list(range(NCORES)),
                               **run_kwargs)

    out = np.empty((B, C, N), dtype=np.float32)
    for core in range(NCORES):
        b, h = divmod(core, 2)
        out[b][:, h * SLAB : (h + 1) * SLAB] = res.results[core]["out"]
    if run_kwargs:
        kernel.last_results = res
    return out


# revision 18
# speedup vs baseline: 1.0249x; 1.0249x over previous
"""Trainium2 Bass kernel for nn_AttentionModule (B=4, C=512, N=4096, CQK=64).

Sharding: 8 cores = (batch b, query-half h). Each core receives x[b] with
columns rotated so that its 2048-query slab is always columns 0:2048 —
attention output for query i depends on the full key set but is invariant
to key permutation, so rotation keeps the program identical across cores.

Per-core pipeline (all on one NeuronCore):
  A) stream x (split across SWDGE-cast and HWDGE+DVE-cast paths), project
     k = Wk x + bk (f32r), q (slab only), and vT[j, c] = (x^T Wv^T)*gamma
     + gamma*bv (produced directly transposed -> no on-chip transposes),
     stored bf16.
  B) per 512-query block: 16 logitsT[j, i] = k^T q matmuls (f32r, j on
     partitions) into 2-bank PSUM groups, one exp per group on ACT -> bf16
     E arena [128, 16384]; denominator = pairwise halving adds (bf16 tree,
     non-destructive level 1) + ones[128,128] matmul (K=128 partition
     reduce); AV accumulated over 32 j-tiles in PSUM (bf16), c-outer with
     rotated j order so each av[c] finishes as its exps land; out =
     AV * recip + x on DVE, emitted inline as each av[c] completes.
"""

import sys

if "/opt/trn_rl_repo" not in sys.path:
    sys.path.insert(0, "/opt/trn_rl_repo")

from contextlib import ExitStack

import numpy as np

import concourse.tile as tile
from concourse import bacc, mybir
from concourse.bass_utils import run_bass_kernel_spmd

B, C, N = 4, 512, 4096
CQK = C // 8
NCORES = 8
SLAB = N // 2            # queries per core
CHUNK = 512              # matmul moving free dim
NCHUNK = N // CHUNK      # 8 column chunks of x
NKT = C // 128           # 4 contraction tiles over input channels
NJT = N // 128           # 32 key tiles
NBLK = SLAB // CHUNK     # 4 query blocks per core
JG = 2                   # j-tiles per logits/exp group
NG = NJT // JG           # 16 groups per block

F32 = mybir.dt.float32
F32R = mybir.dt.float32r
BF16 = mybir.dt.bfloat16

_compiled = None


def _build():
    nc = bacc.Bacc("TRN2", debug=False, num_devices=NCORES)

    x_d = nc.dram_tensor("x", [C, N], F32, kind="ExternalInput").ap()
    wqT_d = nc.dram_tensor("wqT", [C, CQK], F32, kind="ExternalInput").ap()
    wkT_d = nc.dram_tensor("wkT", [C, CQK], F32, kind="ExternalInput").ap()
    wvT_d = nc.dram_tensor("wvT", [C, C], F32, kind="ExternalInput").ap()
    bq_d = nc.dram_tensor("bq", [CQK, 1], F32, kind="ExternalInput").ap()
    bk_d = nc.dram_tensor("bk", [CQK, 1], F32, kind="ExternalInput").ap()
    bvg_d = nc.dram_tensor("bvg", [128, C], F32, kind="ExternalInput").ap()
    ones_d = nc.dram_tensor("ones", [128, 128], F32, kind="ExternalInput").ap()
    out_d = nc.dram_tensor("out", [C, SLAB], F32, kind="ExternalOutput").ap()

    with tile.TileContext(nc) as tc, ExitStack() as ctx:
        consts = ctx.enter_context(tc.tile_pool(name="consts", bufs=1))
        xs_pool = ctx.enter_context(tc.tile_pool(name="xs", bufs=8))
        xf_pool = ctx.enter_context(tc.tile_pool(name="xf", bufs=4))
        qk_pool = ctx.enter_context(tc.tile_pool(name="qk", bufs=1))
        vt_pool = ctx.enter_context(tc.tile_pool(name="vt", bufs=NJT))
        e_pool = ctx.enter_context(tc.tile_pool(name="e", bufs=2))
        sc_pool = ctx.enter_context(tc.tile_pool(name="sc", bufs=1))
        sm_pool = ctx.enter_context(tc.tile_pool(name="sm", bufs=2))
        xr_pool = ctx.enter_context(tc.tile_pool(name="xr", bufs=2))
        o_pool = ctx.enter_context(tc.tile_pool(name="o", bufs=2))
        big_ps = ctx.enter_context(tc.tile_pool(name="bigps", bufs=2, space="PSUM"))
        av_ps = ctx.enter_context(tc.tile_pool(name="avps", bufs=4, space="PSUM"))

        # --- constants (combined single-DMA weight loads) ---
        wk_all = consts.tile([128, NKT * CQK], F32R, tag="wk")
        wq_all = consts.tile([128, NKT * CQK], F32R, tag="wq")
        wv_all = consts.tile([128, NKT * CHUNK], F32R, tag="wv")
        bq = consts.tile([CQK, 1], F32, tag="bq")
        bk = consts.tile([CQK, 1], F32, tag="bk")
        bvg = consts.tile([128, C], F32, tag="bvg")
        ones = consts.tile([128, 128], BF16, tag="ones")
        nc.gpsimd.dma_start(wk_all[:].rearrange("p (k c) -> p k c", k=NKT),
                            wkT_d.rearrange("(k p) c -> p k c", k=NKT))
        nc.gpsimd.dma_start(wq_all[:].rearrange("p (k c) -> p k c", k=NKT),
                            wqT_d.rearrange("(k p) c -> p k c", k=NKT))
        nc.sync.dma_start(bq[:], bq_d[:])
        nc.sync.dma_start(bk[:], bk_d[:])
        wk = [wk_all[:, k * CQK : (k + 1) * CQK] for k in range(NKT)]
        wq = [wq_all[:, k * CQK : (k + 1) * CQK] for k in range(NKT)]
        wv = [wv_all[:, k * CHUNK : (k + 1) * CHUNK] for k in range(NKT)]

        # low half (partitions 0:64) written by projections; high half is a
        # DMA copy so logits matmuls can row-pack two j-tiles per PE pass
        q_sb = qk_pool.tile([128, SLAB], F32R, tag="q")
        k_sb = qk_pool.tile([128, N], F32R, tag="k")
        vt = []  # 32 tiles [128 j, 512 c] bf16

        # --- phase A: projections ---
        for ch in range(NCHUNK):
            cols = slice(ch * CHUNK, (ch + 1) * CHUNK)
            xt = []
            for k in range(NKT):
                t = xs_pool.tile([128, CHUNK], F32R, tag="xs")
                # chunk 0 entirely via HWDGE so PE start waits only on wk;
                # later chunks split across SWDGE-cast and HWDGE+DVE-cast
                if ch > 0 and k % 2 == 0:
                    nc.gpsimd.dma_start(t[:], x_d[k * 128 : (k + 1) * 128, cols])
                else:
                    tf = xf_pool.tile([128, CHUNK], F32, tag="xf")
                    nc.sync.dma_start(tf[:], x_d[k * 128 : (k + 1) * 128, cols])
                    nc.vector.tensor_copy(t[:], tf[:])
                xt.append(t)
            if ch == 0:
                nc.sync.dma_start(bvg[:], bvg_d[:])
                nc.gpsimd.dma_start(ones[:], ones_d[:])
                # wv is first needed by the vT matmuls of chunk 0; loading it
                # here keeps the k/q projections' critical path short
                nc.gpsimd.dma_start(
                    wv_all[:].rearrange("p (k c) -> p k c", k=NKT),
                    wvT_d.rearrange("(k p) c -> p k c", k=NKT))

            k_ps = av_ps.tile([CQK, CHUNK], F32, tag="av")
            for k in range(NKT):
                nc.tensor.matmul(k_ps[:], wk[k], xt[k][:],
                                 start=(k == 0), stop=(k == NKT - 1))
            nc.vector.tensor_scalar_add(k_sb[0:CQK, cols], k_ps[:], bk[:])
            nc.sync.dma_start(k_sb[CQK:128, cols], k_sb[0:CQK, cols])

            if ch < NBLK:
                q_ps = av_ps.tile([CQK, CHUNK], F32, tag="av")
                for k in range(NKT):
                    nc.tensor.matmul(q_ps[:], wq[k], xt[k][:],
                                     start=(k == 0), stop=(k == NKT - 1))
                nc.vector.tensor_scalar_add(q_sb[0:CQK, cols], q_ps[:], bq[:])
                nc.sync.dma_start(q_sb[CQK:128, cols], q_sb[0:CQK, cols])

            for jt in range(4):
                jcols = slice(jt * 128, (jt + 1) * 128)
                v_ps = av_ps.tile([128, C], F32, tag="av")
                for k in range(NKT):
                    nc.tensor.matmul(v_ps[:], xt[k][:, jcols], wv[k],
                                     start=(k == 0), stop=(k == NKT - 1))
                v_t = vt_pool.tile([128, C], BF16, tag="vt")
                nc.vector.tensor_add(v_t[:], v_ps[:], bvg[:])
                vt.append(v_t)

        # --- phase B: attention per query block ---
        # Software pipeline across blocks: emit L[b+1] (logits+exp+tree, which
        # depend only on q/k) before AV[b], so PE never waits on trailing exps
        # at block boundaries.
        H = NJT * CHUNK // 2  # arena half width (8192)

        def emit_L(blk):
            icols = slice(blk * CHUNK, (blk + 1) * CHUNK)
            arena = e_pool.tile([128, NJT * CHUNK], BF16, tag="arena",
                                name=f"arena{blk}")
            scratch = sc_pool.tile([128, H], BF16, tag="scratch",
                                   name=f"scratch{blk}")
            for g in range(NG):
                l_ps = big_ps.tile([128, JG * CHUNK], F32, tag="big",
                                   name=f"lps{blk}_{g}")
                for j in range(JG):
                    jt = g * JG + j
                    # row-pack: even j-tile on array rows 0-63, odd on 64-127;
                    # the two matmuls execute concurrently in the PE array
                    lo, hi = (0, CQK) if j % 2 == 0 else (CQK, 128)
                    nc.tensor.matmul(l_ps[:, j * CHUNK : (j + 1) * CHUNK],
                                     k_sb[lo:hi, jt * 128 : (jt + 1) * 128],
                                     q_sb[lo:hi, icols], start=True, stop=True,
                                     tile_position=(lo, 0))
                nc.scalar.activation(arena[:, g * JG * CHUNK : (g + 1) * JG * CHUNK],
                                     l_ps[:], mybir.ActivationFunctionType.Exp)
                with nc.allow_low_precision(reason="bf16 pairwise exp-sum tree"):
                    if g == NG // 2 - 1:
                        nc.vector.tensor_add(scratch[:, 0 : H // 2],
                                             arena[:, 0 : H // 2],
                                             arena[:, H // 2 : H])
                    elif g == NG - 1:
                        nc.vector.tensor_add(scratch[:, H // 2 : H],
                                             arena[:, H : H + H // 2],
                                             arena[:, H + H // 2 :])
            # finish the halving tree (in place on scratch)
            with nc.allow_low_precision(reason="bf16 pairwise exp-sum tree"):
                w = H // 2
                while w >= CHUNK:
                    nc.vector.tensor_add(scratch[:, 0:w], scratch[:, 0:w],
                                         scratch[:, w : 2 * w])
                    w //= 2
            return arena, scratch

        def emit_AV(blk, arena, scratch):
            icols = slice(blk * CHUNK, (blk + 1) * CHUNK)
            av = [av_ps.tile([128, CHUNK], F32, tag="av", name=f"av{blk}_{i}")
                  for i in range(NKT)]
            recip = sm_pool.tile([128, CHUNK], F32, tag="recip", name=f"rc{blk}")

            def norm_c(c):
                rows = slice(c * 128, (c + 1) * 128)
                xres = xr_pool.tile([128, CHUNK], F32, tag="xr", name=f"xr{blk}_{c}")
                nc.sync.dma_start(xres[:], x_d[rows, icols])
                t = o_pool.tile([128, CHUNK], F32, tag="om", name=f"om{blk}_{c}")
                nc.vector.tensor_mul(t[:], av[c][:], recip[:])
                o = o_pool.tile([128, CHUNK], F32, tag="oo", name=f"oo{blk}_{c}")
                nc.vector.tensor_add(o[:], t[:], xres[:])
                nc.sync.dma_start(out_d[rows, icols], o[:])

            for c in range(NKT):
                for t in range(NJT):
                    jt = (c * (NJT // NKT) + t) % NJT
                    nc.tensor.matmul(av[c][:],
                                     vt[jt][:, c * 128 : (c + 1) * 128],
                                     arena[:, jt * CHUNK : (jt + 1) * CHUNK],
                                     start=(t == 0), stop=(t == NJT - 1))
                if c == 1:
                    # denominator: reduce over partitions, broadcast to all
                    s_ps = big_ps.tile([128, CHUNK], F32, tag="big",
                                       name=f"sps{blk}")
                    nc.tensor.matmul(s_ps[:], ones[:], scratch[:, 0:CHUNK],
                                     start=True, stop=True)
                    nc.vector.reciprocal(recip[:], s_ps[:])
                elif c == 2:
                    norm_c(0)
                elif c == 3:
                    norm_c(1)
            norm_c(2)
            norm_c(3)

        pending = [emit_L(0)]
        for blk in range(NBLK):
            if blk + 1 < NBLK:
                pending.append(emit_L(blk + 1))
            emit_AV(blk, *pending[blk])

    nc.compile()
    return nc


def _get_compiled():
    global _compiled
    if _compiled is None:
        _compiled = _build()
    return _compiled


def kernel(x, Wq, bq, Wk, bk, Wv, bv, gamma, **run_kwargs):
    x = np.asarray(x, dtype=np.float32)
    Wq = np.asarray(Wq, dtype=np.float32)
    bq = np.asarray(bq, dtype=np.float32)
    Wk = np.asarray(Wk, dtype=np.float32)
    bk = np.asarray(bk, dtype=np.float32)
    Wv = np.asarray(Wv, dtype=np.float32)
    bv = np.asarray(bv, dtype=np.float32)
    g = float(np.asarray(gamma).reshape(-1)[0])

    shared = {
        "wqT": np.ascontiguousarray(Wq.T),
        "wkT": np.ascontiguousarray(Wk.T),
        "wvT": np.ascontiguousarray(Wv.T * g),
        "bq": np.ascontiguousarray(bq.reshape(CQK, 1)),
        "bk": np.ascontiguousarray(bk.reshape(CQK, 1)),
        "bvg": np.ascontiguousarray(np.tile((bv * g).reshape(1, C), (128, 1))),
        "ones": np.ones((128, 128), dtype=np.float32),
    }
    in_maps = []
    for core in range(NCORES):
        b, h = divmod(core, 2)
        xb = x[b]
        if h:
            xb = np.concatenate([xb[:, SLAB:], xb[:, :SLAB]], axis=1)
        in_maps.append({"x": np.ascontiguousarray(xb), **shared})

    nc = _get_compiled()
    res = run_bass_kernel_spmd(nc, in_maps, core_ids=list(range(NCORES)),
                               **run_kwargs)

    out = np.empty((B, C, N), dtype=np.float32)
    for core in range(NCORES):
        b, h = divmod(core, 2)
        out[b][:, h * SLAB : (h + 1) * SLAB] = res.results[core]["out"]
    if run_kwargs:
        kernel.last_results = res
    return out


# revision 19
# speedup vs baseline: 1.0476x; 1.0222x over previous
"""Trainium2 Bass kernel for nn_AttentionModule (B=4, C=512, N=4096, CQK=64).

Sharding: 8 cores = (batch b, query-half h). Each core receives x[b] with
columns rotated so that its 2048-query slab is always columns 0:2048 —
attention output for query i depends on the full key set but is invariant
to key permutation, so rotation keeps the program identical across cores.

Per-core pipeline (all on one NeuronCore):
  A) stream x (split across SWDGE-cast and HWDGE+DVE-cast paths), project
     k = Wk x + bk (f32r), q (slab only), and vT[j, c] = (x^T Wv^T)*gamma
     + gamma*bv (produced directly transposed -> no on-chip transposes),
     stored bf16.
  B) per 512-query block: 16 logitsT[j, i] = k^T q matmuls (f32r, j on
     partitions) into 2-bank PSUM groups, one exp per group on ACT -> bf16
     E arena [128, 16384]; denominator = pairwise halving adds (bf16 tree,
     non-destructive level 1) + ones[128,128] matmul (K=128 partition
     reduce); AV accumulated over 32 j-tiles in PSUM (bf16), c-outer with
     rotated j order so each av[c] finishes as its exps land; out =
     AV * recip + x on DVE, emitted inline as each av[c] completes.
"""

import sys

if "/opt/trn_rl_repo" not in sys.path:
    sys.path.insert(0, "/opt/trn_rl_repo")

from contextlib import ExitStack

import numpy as np

import concourse.tile as tile
from concourse import bacc, mybir
from concourse.bass_utils import run_bass_kernel_spmd

B, C, N = 4, 512, 4096
CQK = C // 8
NCORES = 8
SLAB = N // 2            # queries per core
CHUNK = 512              # matmul moving free dim
NCHUNK = N // CHUNK      # 8 column chunks of x
NKT = C // 128           # 4 contraction tiles over input channels
NJT = N // 128           # 32 key tiles
NBLK = SLAB // CHUNK     # 4 query blocks per core
JG = 2                   # j-tiles per logits/exp group
NG = NJT // JG           # 16 groups per block

F32 = mybir.dt.float32
F32R = mybir.dt.float32r
BF16 = mybir.dt.bfloat16

_compiled = None


def _build():
    nc = bacc.Bacc("TRN2", debug=False, num_devices=NCORES)

    x_d = nc.dram_tensor("x", [C, N], F32, kind="ExternalInput").ap()
    wkqT_d = nc.dram_tensor("wkqT", [C, 128], F32, kind="ExternalInput").ap()
    wvT_d = nc.dram_tensor("wvT", [C, C], F32, kind="ExternalInput").ap()
    bkq_d = nc.dram_tensor("bkq", [128, 1], F32, kind="ExternalInput").ap()
    bvg_d = nc.dram_tensor("bvg", [128, C], F32, kind="ExternalInput").ap()
    ones_d = nc.dram_tensor("ones", [128, 128], F32, kind="ExternalInput").ap()
    out_d = nc.dram_tensor("out", [C, SLAB], F32, kind="ExternalOutput").ap()

    with tile.TileContext(nc) as tc, ExitStack() as ctx:
        consts = ctx.enter_context(tc.tile_pool(name="consts", bufs=1))
        xs_pool = ctx.enter_context(tc.tile_pool(name="xs", bufs=8))
        xf_pool = ctx.enter_context(tc.tile_pool(name="xf", bufs=4))
        qk_pool = ctx.enter_context(tc.tile_pool(name="qk", bufs=1))
        vt_pool = ctx.enter_context(tc.tile_pool(name="vt", bufs=NJT))
        e_pool = ctx.enter_context(tc.tile_pool(name="e", bufs=2))
        sc_pool = ctx.enter_context(tc.tile_pool(name="sc", bufs=1))
        sm_pool = ctx.enter_context(tc.tile_pool(name="sm", bufs=2))
        xr_pool = ctx.enter_context(tc.tile_pool(name="xr", bufs=2))
        o_pool = ctx.enter_context(tc.tile_pool(name="o", bufs=2))
        big_ps = ctx.enter_context(tc.tile_pool(name="bigps", bufs=2, space="PSUM"))
        av_ps = ctx.enter_context(tc.tile_pool(name="avps", bufs=4, space="PSUM"))

        # --- constants (combined single-DMA weight loads) ---
        wkq_all = consts.tile([128, NKT * 128], F32R, tag="wkq")
        wv_all = consts.tile([128, NKT * CHUNK], F32R, tag="wv")
        bkq = consts.tile([128, 1], F32, tag="bkq")
        bvg = consts.tile([128, C], F32, tag="bvg")
        ones = consts.tile([128, 128], BF16, tag="ones")
        nc.gpsimd.dma_start(wkq_all[:].rearrange("p (k c) -> p k c", k=NKT),
                            wkqT_d.rearrange("(k p) c -> p k c", k=NKT))
        nc.sync.dma_start(bkq[:], bkq_d[:])
        wkq = [wkq_all[:, k * 128 : (k + 1) * 128] for k in range(NKT)]
        wk = [wkq_all[:, k * 128 : k * 128 + CQK] for k in range(NKT)]
        wv = [wv_all[:, k * CHUNK : (k + 1) * CHUNK] for k in range(NKT)]

        # low half (partitions 0:64) written by projections; high half is a
        # DMA copy so logits matmuls can row-pack two j-tiles per PE pass
        q_sb = qk_pool.tile([128, SLAB], F32R, tag="q")
        k_sb = qk_pool.tile([128, N], F32R, tag="k")
        vt = []  # 32 tiles [128 j, 512 c] bf16

        # --- phase A: projections ---
        for ch in range(NCHUNK):
            cols = slice(ch * CHUNK, (ch + 1) * CHUNK)
            xt = []
            for k in range(NKT):
                t = xs_pool.tile([128, CHUNK], F32R, tag="xs")
                # chunk 0 entirely via HWDGE so PE start waits only on wk;
                # later chunks split across SWDGE-cast and HWDGE+DVE-cast
                if ch > 0 and k % 2 == 0:
                    nc.gpsimd.dma_start(t[:], x_d[k * 128 : (k + 1) * 128, cols])
                else:
                    tf = xf_pool.tile([128, CHUNK], F32, tag="xf")
                    nc.sync.dma_start(tf[:], x_d[k * 128 : (k + 1) * 128, cols])
                    nc.vector.tensor_copy(t[:], tf[:])
                xt.append(t)
            if ch == 0:
                nc.sync.dma_start(bvg[:], bvg_d[:])
                nc.gpsimd.dma_start(ones[:], ones_d[:])
                # wv is first needed by the vT matmuls of chunk 0; loading it
                # here keeps the k/q projections' critical path short
                nc.gpsimd.dma_start(
                    wv_all[:].rearrange("p (k c) -> p k c", k=NKT),
                    wvT_d.rearrange("(k p) c -> p k c", k=NKT))

            if ch < NBLK:
                # k and q share one M=128 matmul pass: k -> psum rows 0:64,
                # q -> rows 64:128 (weights concatenated host-side)
                kq_ps = av_ps.tile([128, CHUNK], F32, tag="av")
                for k in range(NKT):
                    nc.tensor.matmul(kq_ps[:], wkq[k], xt[k][:],
                                     start=(k == 0), stop=(k == NKT - 1))
                nc.vector.tensor_scalar_add(k_sb[0:CQK, cols],
                                            kq_ps[0:CQK, :], bkq[0:CQK])
                nc.vector.tensor_scalar_add(q_sb[CQK:128, cols],
                                            kq_ps[CQK:128, :], bkq[CQK:128])
                nc.sync.dma_start(k_sb[CQK:128, cols], k_sb[0:CQK, cols])
                nc.sync.dma_start(q_sb[0:CQK, cols], q_sb[CQK:128, cols])
            else:
                k_ps = av_ps.tile([CQK, CHUNK], F32, tag="av", name=f"kps{ch}")
                for k in range(NKT):
                    nc.tensor.matmul(k_ps[:], wk[k], xt[k][:],
                                     start=(k == 0), stop=(k == NKT - 1))
                nc.vector.tensor_scalar_add(k_sb[0:CQK, cols], k_ps[:],
                                            bkq[0:CQK])
                nc.sync.dma_start(k_sb[CQK:128, cols], k_sb[0:CQK, cols])

            for jt in range(4):
                jcols = slice(jt * 128, (jt + 1) * 128)
                v_ps = av_ps.tile([128, C], F32, tag="av")
                for k in range(NKT):
                    nc.tensor.matmul(v_ps[:], xt[k][:, jcols], wv[k],
                                     start=(k == 0), stop=(k == NKT - 1))
                v_t = vt_pool.tile([128, C], BF16, tag="vt")
                nc.vector.tensor_add(v_t[:], v_ps[:], bvg[:])
                vt.append(v_t)

        # --- phase B: attention per query block ---
        # Software pipeline across blocks: emit L[b+1] (logits+exp+tree, which
        # depend only on q/k) before AV[b], so PE never waits on trailing exps
        # at block boundaries.
        H = NJT * CHUNK // 2  # arena half width (8192)

        def emit_L(blk):
            icols = slice(blk * CHUNK, (blk + 1) * CHUNK)
            arena = e_pool.tile([128, NJT * CHUNK], BF16, tag="arena",
                                name=f"arena{blk}")
            scratch = sc_pool.tile([128, H], BF16, tag="scratch",
                                   name=f"scratch{blk}")
            for g in range(NG):
                l_ps = big_ps.tile([128, JG * CHUNK], F32, tag="big",
                                   name=f"lps{blk}_{g}")
                for j in range(JG):
                    jt = g * JG + j
                    # row-pack: even j-tile on array rows 0-63, odd on 64-127;
                    # the two matmuls execute concurrently in the PE array
                    lo, hi = (0, CQK) if j % 2 == 0 else (CQK, 128)
                    nc.tensor.matmul(l_ps[:, j * CHUNK : (j + 1) * CHUNK],
                                     k_sb[lo:hi, jt * 128 : (jt + 1) * 128],
                                     q_sb[lo:hi, icols], start=True, stop=True,
                                     tile_position=(lo, 0))
                nc.scalar.activation(arena[:, g * JG * CHUNK : (g + 1) * JG * CHUNK],
                                     l_ps[:], mybir.ActivationFunctionType.Exp)
                with nc.allow_low_precision(reason="bf16 pairwise exp-sum tree"):
                    if g == NG // 2 - 1:
                        nc.vector.tensor_add(scratch[:, 0 : H // 2],
                                             arena[:, 0 : H // 2],
                                             arena[:, H // 2 : H])
                    elif g == NG - 1:
                        nc.vector.tensor_add(scratch[:, H // 2 : H],
                                             arena[:, H : H + H // 2],
                                             arena[:, H + H // 2 :])
            # finish the halving tree (in place on scratch)
            with nc.allow_low_precision(reason="bf16 pairwise exp-sum tree"):
                w = H // 2
                while w >= CHUNK:
                    nc.vector.tensor_add(scratch[:, 0:w], scratch[:, 0:w],
                                         scratch[:, w : 2 * w])
                    w //= 2
            return arena, scratch

        def emit_AV(blk, arena, scratch):
            icols = slice(blk * CHUNK, (blk + 1) * CHUNK)
            corder = [2, 3, 0, 1] if blk == NBLK - 1 else [0, 1, 2, 3]
            av = [av_ps.tile([128, CHUNK], F32, tag="av", name=f"av{blk}_{i}")
                  for i in range(NKT)]
            recip = sm_pool.tile([128, CHUNK], F32, tag="recip", name=f"rc{blk}")

            def norm_c(c):
                rows = slice(c * 128, (c + 1) * 128)
                xres = xr_pool.tile([128, CHUNK], F32, tag="xr", name=f"xr{blk}_{c}")
                nc.sync.dma_start(xres[:], x_d[rows, icols])
                t = o_pool.tile([128, CHUNK], F32, tag="om", name=f"om{blk}_{c}")
                nc.vector.tensor_mul(t[:], av[c][:], recip[:])
                o = o_pool.tile([128, CHUNK], F32, tag="oo", name=f"oo{blk}_{c}")
                nc.vector.tensor_add(o[:], t[:], xres[:])
                nc.sync.dma_start(out_d[rows, icols], o[:])

            for idx, c in enumerate(corder):
                for t in range(NJT):
                    jt = (idx * (NJT // NKT) + t) % NJT
                    nc.tensor.matmul(av[c][:],
                                     vt[jt][:, c * 128 : (c + 1) * 128],
                                     arena[:, jt * CHUNK : (jt + 1) * CHUNK],
                                     start=(t == 0), stop=(t == NJT - 1))
                if idx == 1:
                    # denominator: reduce over partitions, broadcast to all
                    s_ps = big_ps.tile([128, CHUNK], F32, tag="big",
                                       name=f"sps{blk}")
                    nc.tensor.matmul(s_ps[:], ones[:], scratch[:, 0:CHUNK],
                                     start=True, stop=True)
                    nc.vector.reciprocal(recip[:], s_ps[:])
                elif idx == 2:
                    norm_c(corder[0])
                elif idx == 3:
                    norm_c(corder[1])
            norm_c(corder[2])
            norm_c(corder[3])

        pending = [emit_L(0)]
        for blk in range(NBLK):
            if blk + 1 < NBLK:
                pending.append(emit_L(blk + 1))
            emit_AV(blk, *pending[blk])

    nc.compile()
    return nc


def _get_compiled():
    global _compiled
    if _compiled is None:
        _compiled = _build()
    return _compiled


def kernel(x, Wq, bq, Wk, bk, Wv, bv, gamma, **run_kwargs):
    x = np.asarray(x, dtype=np.float32)
    Wq = np.asarray(Wq, dtype=np.float32)
    bq = np.asarray(bq, dtype=np.float32)
    Wk = np.asarray(Wk, dtype=np.float32)
    bk = np.asarray(bk, dtype=np.float32)
    Wv = np.asarray(Wv, dtype=np.float32)
    bv = np.asarray(bv, dtype=np.float32)
    g = float(np.asarray(gamma).reshape(-1)[0])

    shared = {
        "wkqT": np.ascontiguousarray(np.concatenate([Wk.T, Wq.T], axis=1)),
        "wvT": np.ascontiguousarray(Wv.T * g),
        "bkq": np.ascontiguousarray(
            np.concatenate([bk, bq]).reshape(128, 1)),
        "bvg": np.ascontiguousarray(np.tile((bv * g).reshape(1, C), (128, 1))),
        "ones": np.ones((128, 128), dtype=np.float32),
    }
    in_maps = []
    for core in range(NCORES):
        b, h = divmod(core, 2)
        xb = x[b]
        if h:
            xb = np.concatenate([xb[:, SLAB:], xb[:, :SLAB]], axis=1)
        in_maps.append({"x": np.ascontiguousarray(xb), **shared})

    nc = _get_compiled()
    res = run_bass_kernel_spmd(nc, in_maps, core_ids=list(range(NCORES)),
                               **run_kwargs)

    out = np.empty((B, C, N), dtype=np.float32)
    for core in range(NCORES):
        b, h = divmod(core, 2)
        out[b][:, h * SLAB : (h + 1) * SLAB] = res.results[core]["out"]
    if run_kwargs:
        kernel.last_results = res
    return out


# revision 22
# speedup vs baseline: 1.0505x; 1.0027x over previous
"""Trainium2 Bass kernel for nn_AttentionModule (B=4, C=512, N=4096, CQK=64).

Sharding: 8 cores = (batch b, query-half h). Each core receives x[b] with
columns rotated so that its 2048-query slab is always columns 0:2048 —
attention output for query i depends on the full key set but is invariant
to key permutation, so rotation keeps the program identical across cores.

Per-core pipeline (all on one NeuronCore):
  A) stream x (split across SWDGE-cast and HWDGE+DVE-cast paths), project
     k = Wk x + bk (f32r), q (slab only), and vT[j, c] = (x^T Wv^T)*gamma
     + gamma*bv (produced directly transposed -> no on-chip transposes),
     stored bf16.
  B) per 512-query block: 16 logitsT[j, i] = k^T q matmuls (f32r, j on
     partitions) into 2-bank PSUM groups, one exp per group on ACT -> bf16
     E arena [128, 16384]; denominator = pairwise halving adds (bf16 tree,
     non-destructive level 1) + ones[128,128] matmul (K=128 partition
     reduce); AV accumulated over 32 j-tiles in PSUM (bf16), c-outer with
     rotated j order so each av[c] finishes as its exps land; out =
     AV * recip + x on DVE, emitted inline as each av[c] completes.
"""

import sys

if "/opt/trn_rl_repo" not in sys.path:
    sys.path.insert(0, "/opt/trn_rl_repo")

from contextlib import ExitStack

import numpy as np

import concourse.tile as tile
from concourse import bacc, mybir
from concourse.bass_utils import run_bass_kernel_spmd

B, C, N = 4, 512, 4096
CQK = C // 8
NCORES = 8
SLAB = N // 2            # queries per core
CHUNK = 512              # matmul moving free dim
NCHUNK = N // CHUNK      # 8 column chunks of x
NKT = C // 128           # 4 contraction tiles over input channels
NJT = N // 128           # 32 key tiles
NBLK = SLAB // CHUNK     # 4 query blocks per core
JG = 2                   # j-tiles per logits/exp group
NG = NJT // JG           # 16 groups per block

F32 = mybir.dt.float32
F32R = mybir.dt.float32r
BF16 = mybir.dt.bfloat16

_compiled = None


def _build():
    nc = bacc.Bacc("TRN2", debug=False, num_devices=NCORES)

    x_d = nc.dram_tensor("x", [C, N], F32, kind="ExternalInput").ap()
    wkqT_d = nc.dram_tensor("wkqT", [C, 128], F32, kind="ExternalInput").ap()
    wvT_d = nc.dram_tensor("wvT", [C, C], F32, kind="ExternalInput").ap()
    bkq_d = nc.dram_tensor("bkq", [128, 1], F32, kind="ExternalInput").ap()
    bvg_d = nc.dram_tensor("bvg", [128, C], F32, kind="ExternalInput").ap()
    ones_d = nc.dram_tensor("ones", [128, 128], F32, kind="ExternalInput").ap()
    out_d = nc.dram_tensor("out", [C, SLAB], F32, kind="ExternalOutput").ap()

    with tile.TileContext(nc) as tc, ExitStack() as ctx:
        consts = ctx.enter_context(tc.tile_pool(name="consts", bufs=1))
        xs_pool = ctx.enter_context(tc.tile_pool(name="xs", bufs=8))
        xf_pool = ctx.enter_context(tc.tile_pool(name="xf", bufs=4))
        qk_pool = ctx.enter_context(tc.tile_pool(name="qk", bufs=1))
        vt_pool = ctx.enter_context(tc.tile_pool(name="vt", bufs=NJT))
        e_pool = ctx.enter_context(tc.tile_pool(name="e", bufs=2))
        sc_pool = ctx.enter_context(tc.tile_pool(name="sc", bufs=1))
        sm_pool = ctx.enter_context(tc.tile_pool(name="sm", bufs=2))
        xr_pool = ctx.enter_context(tc.tile_pool(name="xr", bufs=2))
        o_pool = ctx.enter_context(tc.tile_pool(name="o", bufs=2))
        big_ps = ctx.enter_context(tc.tile_pool(name="bigps", bufs=2, space="PSUM"))
        av_ps = ctx.enter_context(tc.tile_pool(name="avps", bufs=4, space="PSUM"))

        # --- constants (combined single-DMA weight loads) ---
        wkq_all = consts.tile([128, NKT * 128], F32R, tag="wkq")
        wv_all = consts.tile([128, NKT * CHUNK], F32R, tag="wv")
        bkq = consts.tile([128, 1], F32, tag="bkq")
        bvg = consts.tile([128, C], F32, tag="bvg")
        ones = consts.tile([128, 128], BF16, tag="ones")
        nc.gpsimd.dma_start(wkq_all[:, 0:128], wkqT_d[0:128, :])
        nc.gpsimd.dma_start(
            wkq_all[:, 128:].rearrange("p (k c) -> p k c", k=NKT - 1),
            wkqT_d[128:, :].rearrange("(k p) c -> p k c", k=NKT - 1))
        nc.sync.dma_start(bkq[:], bkq_d[:])
        wkq = [wkq_all[:, k * 128 : (k + 1) * 128] for k in range(NKT)]
        wk = [wkq_all[:, k * 128 : k * 128 + CQK] for k in range(NKT)]
        wv = [wv_all[:, k * CHUNK : (k + 1) * CHUNK] for k in range(NKT)]

        # low half (partitions 0:64) written by projections; high half is a
        # DMA copy so logits matmuls can row-pack two j-tiles per PE pass
        q_sb = qk_pool.tile([128, SLAB], F32R, tag="q")
        k_sb = qk_pool.tile([128, N], F32R, tag="k")
        vt = []  # 32 tiles [128 j, 512 c] bf16

        # --- phase A: projections ---
        for ch in range(NCHUNK):
            cols = slice(ch * CHUNK, (ch + 1) * CHUNK)
            xt = []
            for k in range(NKT):
                t = xs_pool.tile([128, CHUNK], F32R, tag="xs")
                # chunk 0 entirely via HWDGE so PE start waits only on wk;
                # later chunks split across SWDGE-cast and HWDGE+DVE-cast
                if ch > 0 and k % 2 == 0:
                    nc.gpsimd.dma_start(t[:], x_d[k * 128 : (k + 1) * 128, cols])
                else:
                    tf = xf_pool.tile([128, CHUNK], F32, tag="xf")
                    nc.sync.dma_start(tf[:], x_d[k * 128 : (k + 1) * 128, cols])
                    nc.vector.tensor_copy(t[:], tf[:])
                xt.append(t)
            if ch == 0:
                nc.sync.dma_start(bvg[:], bvg_d[:])
                nc.gpsimd.dma_start(ones[:], ones_d[:])
                # wv is first needed by the vT matmuls of chunk 0; loading it
                # here keeps the k/q projections' critical path short
                nc.gpsimd.dma_start(
                    wv_all[:].rearrange("p (k c) -> p k c", k=NKT),
                    wvT_d.rearrange("(k p) c -> p k c", k=NKT))

            if ch < NBLK:
                # k and q share one M=128 matmul pass: k -> psum rows 0:64,
                # q -> rows 64:128 (weights concatenated host-side)
                kq_ps = av_ps.tile([128, CHUNK], F32, tag="av")
                for k in range(NKT):
                    nc.tensor.matmul(kq_ps[:], wkq[k], xt[k][:],
                                     start=(k == 0), stop=(k == NKT - 1))
                nc.vector.tensor_scalar_add(k_sb[0:CQK, cols],
                                            kq_ps[0:CQK, :], bkq[0:CQK])
                nc.vector.tensor_scalar_add(q_sb[CQK:128, cols],
                                            kq_ps[CQK:128, :], bkq[CQK:128])
                nc.sync.dma_start(k_sb[CQK:128, cols], k_sb[0:CQK, cols])
                nc.sync.dma_start(q_sb[0:CQK, cols], q_sb[CQK:128, cols])
            else:
                k_ps = av_ps.tile([CQK, CHUNK], F32, tag="av", name=f"kps{ch}")
                for k in range(NKT):
                    nc.tensor.matmul(k_ps[:], wk[k], xt[k][:],
                                     start=(k == 0), stop=(k == NKT - 1))
                nc.vector.tensor_scalar_add(k_sb[0:CQK, cols], k_ps[:],
                                            bkq[0:CQK])
                nc.sync.dma_start(k_sb[CQK:128, cols], k_sb[0:CQK, cols])

            for jt in range(4):
                jcols = slice(jt * 128, (jt + 1) * 128)
                v_ps = av_ps.tile([128, C], F32, tag="av")
                for k in range(NKT):
                    nc.tensor.matmul(v_ps[:], xt[k][:, jcols], wv[k],
                                     start=(k == 0), stop=(k == NKT - 1))
                v_t = vt_pool.tile([128, C], BF16, tag="vt")
                nc.vector.tensor_add(v_t[:], v_ps[:], bvg[:])
                vt.append(v_t)

        # --- phase B: attention per query block ---
        # Software pipeline across blocks: emit L[b+1] (logits+exp+tree, which
        # depend only on q/k) before AV[b], so PE never waits on trailing exps
        # at block boundaries.
        H = NJT * CHUNK // 2  # arena half width (8192)

        def emit_L(blk):
            icols = slice(blk * CHUNK, (blk + 1) * CHUNK)
            arena = e_pool.tile([128, NJT * CHUNK], BF16, tag="arena",
                                name=f"arena{blk}")
            scratch = sc_pool.tile([128, H], BF16, tag="scratch",
                                   name=f"scratch{blk}")
            for g in range(NG):
                l_ps = big_ps.tile([128, JG * CHUNK], F32, tag="big",
                                   name=f"lps{blk}_{g}")
                for j in range(JG):
                    jt = g * JG + j
                    # row-pack: even j-tile on array rows 0-63, odd on 64-127;
                    # the two matmuls execute concurrently in the PE array
                    lo, hi = (0, CQK) if j % 2 == 0 else (CQK, 128)
                    nc.tensor.matmul(l_ps[:, j * CHUNK : (j + 1) * CHUNK],
                                     k_sb[lo:hi, jt * 128 : (jt + 1) * 128],
                                     q_sb[lo:hi, icols], start=True, stop=True,
                                     tile_position=(lo, 0))
                nc.scalar.activation(arena[:, g * JG * CHUNK : (g + 1) * JG * CHUNK],
                                     l_ps[:], mybir.ActivationFunctionType.Exp)
                with nc.allow_low_precision(reason="bf16 pairwise exp-sum tree"):
                    if g == NG // 2 - 1:
                        nc.vector.tensor_add(scratch[:, 0 : H // 2],
                                             arena[:, 0 : H // 2],
                                             arena[:, H // 2 : H])
                    elif g == NG - 1:
                        nc.vector.tensor_add(scratch[:, H // 2 : H],
                                             arena[:, H : H + H // 2],
                                             arena[:, H + H // 2 :])
            # finish the halving tree (in place on scratch)
            with nc.allow_low_precision(reason="bf16 pairwise exp-sum tree"):
                w = H // 2
                while w >= CHUNK:
                    nc.vector.tensor_add(scratch[:, 0:w], scratch[:, 0:w],
                                         scratch[:, w : 2 * w])
                    w //= 2
            return arena, scratch

        def emit_AV(blk, arena, scratch):
            icols = slice(blk * CHUNK, (blk + 1) * CHUNK)
            corder = [2, 3, 0, 1] if blk == NBLK - 1 else [0, 1, 2, 3]
            av = [av_ps.tile([128, CHUNK], F32, tag="av", name=f"av{blk}_{i}")
                  for i in range(NKT)]
            recip = sm_pool.tile([128, CHUNK], F32, tag="recip", name=f"rc{blk}")

            def norm_c(c):
                rows = slice(c * 128, (c + 1) * 128)
                xres = xr_pool.tile([128, CHUNK], F32, tag="xr", name=f"xr{blk}_{c}")
                nc.sync.dma_start(xres[:], x_d[rows, icols])
                t = o_pool.tile([128, CHUNK], F32, tag="om", name=f"om{blk}_{c}")
                nc.vector.tensor_mul(t[:], av[c][:], recip[:])
                o = o_pool.tile([128, CHUNK], F32, tag="oo", name=f"oo{blk}_{c}")
                nc.vector.tensor_add(o[:], t[:], xres[:])
                nc.sync.dma_start(out_d[rows, icols], o[:])

            for idx, c in enumerate(corder):
                for t in range(NJT):
                    jt = (idx * (NJT // NKT) + t) % NJT
                    nc.tensor.matmul(av[c][:],
                                     vt[jt][:, c * 128 : (c + 1) * 128],
                                     arena[:, jt * CHUNK : (jt + 1) * CHUNK],
                                     start=(t == 0), stop=(t == NJT - 1))
                if idx == 1:
                    # denominator: reduce over partitions, broadcast to all
                    s_ps = big_ps.tile([128, CHUNK], F32, tag="big",
                                       name=f"sps{blk}")
                    nc.tensor.matmul(s_ps[:], ones[:], scratch[:, 0:CHUNK],
                                     start=True, stop=True)
                    nc.vector.reciprocal(recip[:], s_ps[:])
                elif idx == 2:
                    norm_c(corder[0])
                elif idx == 3:
                    norm_c(corder[1])
                    norm_c(corder[2])
            norm_c(corder[3])

        pending = [emit_L(0)]
        for blk in range(NBLK):
            if blk + 1 < NBLK:
                pending.append(emit_L(blk + 1))
            emit_AV(blk, *pending[blk])

    nc.compile()
    return nc


def _get_compiled():
    global _compiled
    if _compiled is None:
        _compiled = _build()
    return _compiled


def kernel(x, Wq, bq, Wk, bk, Wv, bv, gamma, **run_kwargs):
    x = np.asarray(x, dtype=np.float32)
    Wq = np.asarray(Wq, dtype=np.float32)
    bq = np.asarray(bq, dtype=np.float32)
    Wk = np.asarray(Wk, dtype=np.float32)
    bk = np.asarray(bk, dtype=np.float32)
    Wv = np.asarray(Wv, dtype=np.float32)
    bv = np.asarray(bv, dtype=np.float32)
    g = float(np.asarray(gamma).reshape(-1)[0])

    shared = {
        "wkqT": np.ascontiguousarray(np.concatenate([Wk.T, Wq.T], axis=1)),
        "wvT": np.ascontiguousarray(Wv.T * g),
        "bkq": np.ascontiguousarray(
            np.concatenate([bk, bq]).reshape(128, 1)),
        "bvg": np.ascontiguousarray(np.tile((bv * g).reshape(1, C), (128, 1))),
        "ones": np.ones((128, 128), dtype=np.float32),
    }
    in_maps = []
    for core in range(NCORES):
        b, h = divmod(core, 2)
        xb = x[b]
        if h:
            xb = np.concatenate([xb[:, SLAB:], xb[:, :SLAB]], axis=1)
        in_maps.append({"x": np.ascontiguousarray(xb), **shared})

    nc = _get_compiled()
    res = run_bass_kernel_spmd(nc, in_maps, core_ids=list(range(NCORES)),
                               **run_kwargs)

    out = np.empty((B, C, N), dtype=np.float32)
    for core in range(NCORES):
        b, h = divmod(core, 2)
        out[b][:, h * SLAB : (h + 1) * SLAB] = res.results[core]["out"]
    if run_kwargs:
        kernel.last_results = res
    return out
